# revision 13
# baseline (speedup 1.0000x reference)
"""BayesianLinear forward kernel for 8x Trainium2 NeuronCores.

out[b,o] = sum_i (mu[o,i] + std[o,i]*eps_w[b,o,i]) * x[b,i]
           + bias_mu[o] + bias_std[o]*eps_b[b,o]

Shapes (full): x (1024,512) f32, eps_w (1024,512,512) f32, eps_b (1024,512) f32,
weight_mu/logvar (512,512) f32, bias_mu/logvar (512,) f32 -> out (1024,512) f32.

Strategy: data-parallel over batch (128 rows/core).  The eps_w stream is the
cost driver (memory-bound).  Two host-side layout/precision choices collapse
the device dataflow:
  * eps_w is cast to bf16 (rel-err budget 2e-2 admits ~6e-3) and transposed
    to [b, i, o] per core shard -> HBM traffic halves to 64 MiB/core and the
    16 per-row PE transposes disappear entirely.
  * x / weight_mu / weight_logvar are uploaded pre-transposed ([i, b]/[i, o]),
    so no on-device preamble transposes either.
Device per eps chunk (RPD batch rows, i on partitions as i = 4p + ki):
  1. DMA chunk (RPD x 512 x 512 bf16) -> SBUF, 4 KiB contiguous per (row,
     partition); chunks alternate between the two HWDGE rings (sync/scalar).
  2. One VectorE tensor_tensor multiplies the chunk by stdT (bf16 2x mode):
     t' = epsT * stdT.
  3. Per row, 4 TensorE matvecs (lhsT = bf16 x-column, rhs = t' k-chunk)
     accumulate out2[b,:] into a PSUM row.
  4. ScalarE copies the PSUM row into a partition-0 staging buffer; every 16
     rows one SBUF->SBUF DMA (gpsimd/SWDGE ring) scatters it to 16 partitions
     of the gather tile G.
Once per core: out1 = x @ mu^T + bias_mu via 4 batched matmuls + K=1 bias
matmuls into PSUM; U = out1 + bias_mu + bias_std*eps_b on VectorE.
Final: out = G + U every 32 rows, DMA to HBM on the gpsimd ring.

Engine budgets per core (128 rows): DMA ~190 us (bound), DVE ~140 us,
PE ~110 us, ACT ~55 us.  Measured steady-state: ~191-208 us/iter
(vs 519 us baseline), ~326-343 GB/s/core of the ~358 GB/s HBM cap.
"""

import os
import sys

import numpy as np

for _p in ("/opt/trn_rl_repo", "/root/.axon_site/_ro/trn_rl_repo"):
    if os.path.isdir(_p) and _p not in sys.path:
        sys.path.insert(0, _p)

import ml_dtypes  # noqa: E402

from concourse import bacc, mybir  # noqa: E402
from concourse import tile  # noqa: E402
from concourse.bass_utils import run_bass_kernel_spmd  # noqa: E402

P = 128          # partitions
I = 512          # in_features
O = 512          # out_features
B_FULL = 1024    # full batch
N_CORES = 8
B = B_FULL // N_CORES   # batch rows per core
KI = I // P      # i-chunks (i = KI*p + ki)
F32 = mybir.dt.float32
BF16 = mybir.dt.bfloat16

RPD = int(os.environ.get("K_RPD", "4"))  # batch rows per eps DMA chunk
EPS_BUFS = int(os.environ.get("K_EPS_BUFS", "5"))
TP_BUFS = int(os.environ.get("K_TP_BUFS", "2"))
SREP = min(RPD, 4)  # stdT bf16 replication factor (SBUF cap)
N_RINGS = int(os.environ.get("K_RINGS", "2"))  # eps DMA rings (2=HWDGE, 3=+gpsimd)
STAGE_CHUNK = int(os.environ.get("K_STAGE", "8"))  # b-rows per stage buffer


def _build_program():
    nc = bacc.Bacc("TRN2", target_bir_lowering=False, debug=False)

    # eps cast to bf16 and pre-swizzled on host into the exact SBUF tile
    # layout [chunk, p, (r ki o)] (b = chunk*RPD+r, i = KI*p+ki), so each
    # chunk DMA is one contiguous 16 KiB run per partition; x/mu/lv are
    # pre-transposed on host ([i, b] / [i, o]).
    epsT_s = nc.dram_tensor("epsT_s", [B // RPD, P, RPD * KI * O], BF16,
                            kind="ExternalInput")
    xT_s = nc.dram_tensor("xT_s", [I, B], F32, kind="ExternalInput")
    muT_d = nc.dram_tensor("muT_d", [I, O], F32, kind="ExternalInput")
    lvT_d = nc.dram_tensor("lvT_d", [I, O], F32, kind="ExternalInput")
    eps_b_s = nc.dram_tensor("eps_b_s", [B, O], F32, kind="ExternalInput")
    b_mu = nc.dram_tensor("b_mu", [1, O], F32, kind="ExternalInput")
    b_lv = nc.dram_tensor("b_lv", [1, O], F32, kind="ExternalInput")
    out_s = nc.dram_tensor("out_s", [B, O], F32, kind="ExternalOutput")

    # Static PSUM: 4 single-bank matvec-row tensors (b % 4) + 2 preamble
    # banks (out1 accumulator, bias_std broadcast).
    prow_st = [nc.alloc_psum_tensor(f"prow{j}", [P, O], F32) for j in range(4)]
    ps_u_st = nc.alloc_psum_tensor("ps_u", [P, O], F32)
    ps_b_st = nc.alloc_psum_tensor("ps_b", [P, O], F32)

    with tile.TileContext(nc) as tc:
        with (
            tc.tile_pool(name="consts", bufs=1) as consts,
            tc.tile_pool(name="eps_pool", bufs=EPS_BUFS) as eps_pool,
            tc.tile_pool(name="tp_pool", bufs=TP_BUFS) as tp_pool,
            tc.tile_pool(name="stage_pool", bufs=2) as stage_pool,
        ):
            # ---- constants / preamble ----------------------------------
            # All preamble DMAs ride the gpsimd (SWDGE) ring so both HWDGE
            # rings are free for the eps stream from t=0.
            lv_sb = consts.tile([P, KI * O], F32)     # [p, ki*O+o] = lv[o, 4p+ki]
            std_sb = consts.tile([P, KI * O], F32)    # exp(0.5*lv)
            stdT_bf = consts.tile([P, SREP * KI * O], BF16)  # bf16, repeated SREP x
            mu_sb = consts.tile([P, KI * O], F32)
            xT_sb = consts.tile([P, KI * B], F32)     # [p, ki*B+b] = x[b, 4p+ki]
            xT_bf = consts.tile([P, KI * B], BF16)
            epsb_sb = consts.tile([P, O], F32)        # [b, o]
            bmu_row = consts.tile([1, O], F32)
            blv_row = consts.tile([1, O], F32)
            bstd_row = consts.tile([1, O], F32)
            ones_col = consts.tile([1, P], F32)
            U = consts.tile([P, O], F32)              # out1 + bias terms, [b, o]
            G = consts.tile([P, O], F32)              # gathered eps-term rows
            final_sb = consts.tile([P, O], F32)

            nc.gpsimd.dma_start(
                out=lv_sb[:].rearrange("p (ki o) -> p ki o", ki=KI),
                in_=lvT_d.ap().rearrange("(p ki) o -> p ki o", p=P),
            )
            nc.gpsimd.dma_start(
                out=mu_sb[:].rearrange("p (ki o) -> p ki o", ki=KI),
                in_=muT_d.ap().rearrange("(p ki) o -> p ki o", p=P),
            )
            nc.gpsimd.dma_start(
                out=xT_sb[:].rearrange("p (ki b) -> p ki b", ki=KI),
                in_=xT_s.ap().rearrange("(p ki) b -> p ki b", p=P),
            )
            nc.gpsimd.dma_start(out=epsb_sb[:], in_=eps_b_s.ap())
            nc.gpsimd.dma_start(out=bmu_row[:], in_=b_mu.ap())
            nc.gpsimd.dma_start(out=blv_row[:], in_=b_lv.ap())
            nc.vector.memset(ones_col[:], 1.0)

            # std = exp(0.5 * logvar) then cast to bf16, replicated RPD x so
            # one tensor_tensor covers a whole RPD-row eps chunk.
            nc.scalar.activation(std_sb[:], lv_sb[:],
                                 mybir.ActivationFunctionType.Exp, scale=0.5)
            nc.scalar.activation(bstd_row[:], blv_row[:],
                                 mybir.ActivationFunctionType.Exp, scale=0.5)
            for r in range(SREP):
                nc.vector.tensor_copy(
                    out=stdT_bf[:, r * KI * O:(r + 1) * KI * O], in_=std_sb[:])
            nc.vector.tensor_copy(out=xT_bf[:], in_=xT_sb[:])

            # out1[b,o] = sum_i x[b,i]*mu[o,i]  (+ bias_mu via K=1 matmul)
            ps_u = ps_u_st.ap()
            for k in range(KI):
                nc.tensor.matmul(
                    out=ps_u,
                    lhsT=xT_sb[:, k * B:(k + 1) * B],
                    rhs=mu_sb[:, k * O:(k + 1) * O],
                    start=(k == 0), stop=False,
                )
            nc.tensor.matmul(out=ps_u, lhsT=ones_col[:], rhs=bmu_row[:],
                             start=False, stop=True)

            # broadcast bias_std across partitions, then
            # U = out1 + bias_mu + bias_std * eps_b
            ps_b = ps_b_st.ap()
            nc.tensor.matmul(out=ps_b, lhsT=ones_col[:], rhs=bstd_row[:],
                             start=True, stop=True)
            nc.vector.tensor_tensor(out=U[:], in0=epsb_sb[:], in1=ps_b,
                                    op=mybir.AluOpType.mult)
            nc.vector.tensor_tensor(out=U[:], in0=U[:], in1=ps_u,
                                    op=mybir.AluOpType.add)

            # ---- main loop over batch rows -----------------------------
            def emit_main(_iv=None):
              stage = None
              n_loop = int(os.environ.get("KERNEL_NB", B))
              for b0 in range(0, n_loop, RPD):
                eps_t = eps_pool.tile([P, RPD * KI * O], BF16, tag="eps")
                rings = [nc.sync, nc.scalar, nc.gpsimd][:N_RINGS]
                eng = rings[(b0 // RPD) % len(rings)]
                eng.dma_start(out=eps_t[:], in_=epsT_s.ap()[b0 // RPD])

                # t' = epsT * stdT, bf16 2x mode, SREP rows per tensor_tensor
                t_p = tp_pool.tile([P, RPD * KI * O], BF16, tag="tp")
                for s0 in range(0, RPD, SREP):
                    sl = slice(s0 * KI * O, (s0 + SREP) * KI * O)
                    nc.vector.tensor_tensor(out=t_p[:, sl], in0=eps_t[:, sl],
                                            in1=stdT_bf[:],
                                            op=mybir.AluOpType.mult)

                for r in range(RPD):
                    b = b0 + r
                    prow = prow_st[b % 4].ap()[:1, :]
                    for k in range(KI):
                        nc.tensor.matmul(
                            out=prow,
                            lhsT=xT_bf[:, k * B + b: k * B + b + 1],
                            rhs=t_p[:, (r * KI + k) * O:(r * KI + k + 1) * O],
                            start=(k == 0), stop=(k == KI - 1),
                        )

                    ci = b % STAGE_CHUNK
                    if ci == 0:
                        stage = stage_pool.tile([1, STAGE_CHUNK * O], F32,
                                                tag="stage")
                    nc.scalar.copy(
                        out=stage[0:1, ci * O:(ci + 1) * O], in_=prow)
                    if ci == STAGE_CHUNK - 1:
                        g0 = b - (STAGE_CHUNK - 1)
                        nc.gpsimd.dma_start(
                            out=G[g0:g0 + STAGE_CHUNK, :],
                            in_=stage[0:1, :],
                        )
                    if b % 32 == 31:
                        f0 = b - 31
                        nc.vector.tensor_tensor(
                            out=final_sb[f0:f0 + 32, :],
                            in0=G[f0:f0 + 32, :],
                            in1=U[f0:f0 + 32, :],
                            op=mybir.AluOpType.add,
                        )
                        nc.gpsimd.dma_start(
                            out=out_s.ap()[f0:f0 + 32, :],
                            in_=final_sb[f0:f0 + 32, :],
                        )

            repeat = int(os.environ.get("KERNEL_REPEAT", "0"))
            if repeat > 1:
                with tc.For_i(0, repeat, 1):
                    emit_main()
            else:
                emit_main()

    nc.compile()
    return nc


_NC = None


def _get_program():
    global _NC
    if _NC is None:
        _NC = _build_program()
    return _NC


def _prep_full(inputs):
    """Host-side layout/precision prep shared by kernel() and test harnesses."""
    x = np.asarray(inputs["x"], dtype=np.float32)
    eps_w = np.asarray(inputs["eps_w"], dtype=np.float32)
    eps_b = np.asarray(inputs["eps_b"], dtype=np.float32)
    w_mu = np.asarray(inputs["weight_mu"], dtype=np.float32)
    w_lv = np.asarray(inputs["weight_logvar"], dtype=np.float32)
    b_mu = np.asarray(inputs["bias_mu"], dtype=np.float32).reshape(1, O)
    b_lv = np.asarray(inputs["bias_logvar"], dtype=np.float32).reshape(1, O)

    eps_bf = eps_w.astype(ml_dtypes.bfloat16)          # (B_FULL, O, I)
    xT = np.ascontiguousarray(x.T)                     # (I, B_FULL)
    muT = np.ascontiguousarray(w_mu.T)                 # (I, O)
    lvT = np.ascontiguousarray(w_lv.T)                 # (I, O)
    return eps_bf, xT, muT, lvT, eps_b, b_mu, b_lv


def _core_maps(eps_bf, xT, muT, lvT, eps_b, b_mu, b_lv):
    in_maps = []
    for ci in range(N_CORES):
        sl = slice(ci * B, (ci + 1) * B)
        # (B, O, I) -> [C, RPD, O, P, KI] -> [C, P, RPD, KI, O] in one copy
        swz = np.ascontiguousarray(
            eps_bf[sl].reshape(B // RPD, RPD, O, P, KI)
            .transpose(0, 3, 1, 4, 2)).reshape(B // RPD, P, RPD * KI * O)
        in_maps.append({
            "epsT_s": swz,
            "xT_s": np.ascontiguousarray(xT[:, sl]),  # (I, B)
            "muT_d": muT,
            "lvT_d": lvT,
            "eps_b_s": np.ascontiguousarray(eps_b[sl]),
            "b_mu": b_mu,
            "b_lv": b_lv,
        })
    return in_maps


def kernel(**inputs) -> np.ndarray:
    in_maps = _core_maps(*_prep_full(inputs))
    nc = _get_program()
    res = run_bass_kernel_spmd(nc, in_maps, core_ids=list(range(N_CORES)))
    out = np.concatenate([res.results[ci]["out_s"] for ci in range(N_CORES)],
                         axis=0)
    return out.astype(np.float32)


# revision 14
# speedup vs baseline: 1.0807x; 1.0807x over previous
"""BayesianLinear forward kernel for 8x Trainium2 NeuronCores.

out[b,o] = sum_i (mu[o,i] + std[o,i]*eps_w[b,o,i]) * x[b,i]
           + bias_mu[o] + bias_std[o]*eps_b[b,o]

Shapes (full): x (1024,512) f32, eps_w (1024,512,512) f32, eps_b (1024,512) f32,
weight_mu/logvar (512,512) f32, bias_mu/logvar (512,) f32 -> out (1024,512) f32.

Strategy: data-parallel over batch (128 rows/core).  The eps_w stream is the
cost driver (memory-bound).  Two host-side layout/precision choices collapse
the device dataflow:
  * eps_w is cast to bf16 (rel-err budget 2e-2 admits ~6e-3) and transposed
    to [b, i, o] per core shard -> HBM traffic halves to 64 MiB/core and the
    16 per-row PE transposes disappear entirely.
  * x / weight_mu / weight_logvar are uploaded pre-transposed ([i, b]/[i, o]),
    so no on-device preamble transposes either.
Device per eps chunk (RPD batch rows, i on partitions as i = 4p + ki):
  1. DMA chunk (RPD x 512 x 512 bf16) -> SBUF, 4 KiB contiguous per (row,
     partition); chunks alternate between the two HWDGE rings (sync/scalar).
  2. One VectorE tensor_tensor multiplies the chunk by stdT (bf16 2x mode):
     t' = epsT * stdT.
  3. Per row, 4 TensorE matvecs (lhsT = bf16 x-column, rhs = t' k-chunk)
     accumulate out2[b,:] into a PSUM row.
  4. ScalarE copies the PSUM row into a partition-0 staging buffer; every 16
     rows one SBUF->SBUF DMA (gpsimd/SWDGE ring) scatters it to 16 partitions
     of the gather tile G.
Once per core: out1 = x @ mu^T + bias_mu via 4 batched matmuls + K=1 bias
matmuls into PSUM; U = out1 + bias_mu + bias_std*eps_b on VectorE.
Final: out = G + U every 32 rows, DMA to HBM on the gpsimd ring.

Engine budgets per core (128 rows): DMA ~190 us (bound), DVE ~140 us,
PE ~110 us, ACT ~55 us.  Measured steady-state: ~191-208 us/iter
(vs 519 us baseline), ~326-343 GB/s/core of the ~358 GB/s HBM cap.
"""

import os
import sys

import numpy as np

for _p in ("/opt/trn_rl_repo", "/root/.axon_site/_ro/trn_rl_repo"):
    if os.path.isdir(_p) and _p not in sys.path:
        sys.path.insert(0, _p)

import ml_dtypes  # noqa: E402

from concourse import bacc, mybir  # noqa: E402
from concourse import tile  # noqa: E402
from concourse.bass_utils import run_bass_kernel_spmd  # noqa: E402

P = 128          # partitions
I = 512          # in_features
O = 512          # out_features
B_FULL = 1024    # full batch
N_CORES = 8
B = B_FULL // N_CORES   # batch rows per core
KI = I // P      # i-chunks (i = KI*p + ki)
F32 = mybir.dt.float32
BF16 = mybir.dt.bfloat16

RPD = int(os.environ.get("K_RPD", "4"))  # batch rows per eps DMA chunk
EPS_BUFS = int(os.environ.get("K_EPS_BUFS", "5"))
TP_BUFS = int(os.environ.get("K_TP_BUFS", "2"))
SREP = min(RPD, 4)  # stdT bf16 replication factor (SBUF cap)
N_RINGS = int(os.environ.get("K_RINGS", "2"))  # eps DMA rings (2=HWDGE, 3=+gpsimd)
STAGE_CHUNK = int(os.environ.get("K_STAGE", "8"))  # b-rows per stage buffer


def _build_program():
    nc = bacc.Bacc("TRN2", target_bir_lowering=False, debug=False)

    # eps transposed to [b, i, o] and cast to bf16 on host; x/mu/lv
    # pre-transposed on host ([i, b] / [i, o]).
    epsT_s = nc.dram_tensor("epsT_s", [B, I, O], BF16, kind="ExternalInput")
    xT_s = nc.dram_tensor("xT_s", [I, B], F32, kind="ExternalInput")
    muT_d = nc.dram_tensor("muT_d", [I, O], F32, kind="ExternalInput")
    lvT_d = nc.dram_tensor("lvT_d", [I, O], F32, kind="ExternalInput")
    eps_b_s = nc.dram_tensor("eps_b_s", [B, O], F32, kind="ExternalInput")
    b_mu = nc.dram_tensor("b_mu", [1, O], F32, kind="ExternalInput")
    b_lv = nc.dram_tensor("b_lv", [1, O], F32, kind="ExternalInput")
    out_s = nc.dram_tensor("out_s", [B, O], F32, kind="ExternalOutput")

    # Static PSUM: 4 single-bank matvec-row tensors (b % 4) + 2 preamble
    # banks (out1 accumulator, bias_std broadcast).
    prow_st = [nc.alloc_psum_tensor(f"prow{j}", [P, O], F32) for j in range(4)]
    ps_u_st = nc.alloc_psum_tensor("ps_u", [P, O], F32)
    ps_b_st = nc.alloc_psum_tensor("ps_b", [P, O], F32)

    with tile.TileContext(nc) as tc:
        with (
            tc.tile_pool(name="consts", bufs=1) as consts,
            tc.tile_pool(name="eps_pool", bufs=EPS_BUFS) as eps_pool,
            tc.tile_pool(name="tp_pool", bufs=TP_BUFS) as tp_pool,
            tc.tile_pool(name="stage_pool", bufs=2) as stage_pool,
        ):
            # ---- constants / preamble ----------------------------------
            # All preamble DMAs ride the gpsimd (SWDGE) ring so both HWDGE
            # rings are free for the eps stream from t=0.
            lv_sb = consts.tile([P, KI * O], F32)     # [p, ki*O+o] = lv[o, 4p+ki]
            std_sb = consts.tile([P, KI * O], F32)    # exp(0.5*lv)
            stdT_bf = consts.tile([P, SREP * KI * O], BF16)  # bf16, repeated SREP x
            mu_sb = consts.tile([P, KI * O], F32)
            xT_sb = consts.tile([P, KI * B], F32)     # [p, ki*B+b] = x[b, 4p+ki]
            xT_bf = consts.tile([P, KI * B], BF16)
            epsb_sb = consts.tile([P, O], F32)        # [b, o]
            bmu_row = consts.tile([1, O], F32)
            blv_row = consts.tile([1, O], F32)
            bstd_row = consts.tile([1, O], F32)
            ones_col = consts.tile([1, P], F32)
            U = consts.tile([P, O], F32)              # out1 + bias terms, [b, o]
            G = consts.tile([P, O], F32)              # gathered eps-term rows
            final_sb = consts.tile([P, O], F32)

            nc.gpsimd.dma_start(
                out=lv_sb[:].rearrange("p (ki o) -> p ki o", ki=KI),
                in_=lvT_d.ap().rearrange("(p ki) o -> p ki o", p=P),
            )
            nc.gpsimd.dma_start(
                out=mu_sb[:].rearrange("p (ki o) -> p ki o", ki=KI),
                in_=muT_d.ap().rearrange("(p ki) o -> p ki o", p=P),
            )
            nc.gpsimd.dma_start(
                out=xT_sb[:].rearrange("p (ki b) -> p ki b", ki=KI),
                in_=xT_s.ap().rearrange("(p ki) b -> p ki b", p=P),
            )
            nc.gpsimd.dma_start(out=epsb_sb[:], in_=eps_b_s.ap())
            nc.gpsimd.dma_start(out=bmu_row[:], in_=b_mu.ap())
            nc.gpsimd.dma_start(out=blv_row[:], in_=b_lv.ap())
            nc.vector.memset(ones_col[:], 1.0)

            # std = exp(0.5 * logvar) then cast to bf16, replicated RPD x so
            # one tensor_tensor covers a whole RPD-row eps chunk.
            nc.scalar.activation(std_sb[:], lv_sb[:],
                                 mybir.ActivationFunctionType.Exp, scale=0.5)
            nc.scalar.activation(bstd_row[:], blv_row[:],
                                 mybir.ActivationFunctionType.Exp, scale=0.5)
            for r in range(SREP):
                nc.vector.tensor_copy(
                    out=stdT_bf[:, r * KI * O:(r + 1) * KI * O], in_=std_sb[:])
            nc.vector.tensor_copy(out=xT_bf[:], in_=xT_sb[:])

            # out1[b,o] = sum_i x[b,i]*mu[o,i]  (+ bias_mu via K=1 matmul)
            ps_u = ps_u_st.ap()
            for k in range(KI):
                nc.tensor.matmul(
                    out=ps_u,
                    lhsT=xT_sb[:, k * B:(k + 1) * B],
                    rhs=mu_sb[:, k * O:(k + 1) * O],
                    start=(k == 0), stop=False,
                )
            nc.tensor.matmul(out=ps_u, lhsT=ones_col[:], rhs=bmu_row[:],
                             start=False, stop=True)

            # broadcast bias_std across partitions, then
            # U = out1 + bias_mu + bias_std * eps_b
            ps_b = ps_b_st.ap()
            nc.tensor.matmul(out=ps_b, lhsT=ones_col[:], rhs=bstd_row[:],
                             start=True, stop=True)
            nc.vector.tensor_tensor(out=U[:], in0=epsb_sb[:], in1=ps_b,
                                    op=mybir.AluOpType.mult)
            nc.vector.tensor_tensor(out=U[:], in0=U[:], in1=ps_u,
                                    op=mybir.AluOpType.add)

            # ---- main loop over batch rows -----------------------------
            def emit_main(_iv=None):
              stage = None
              n_loop = int(os.environ.get("KERNEL_NB", B))
              for b0 in range(0, n_loop, RPD):
                eps_t = eps_pool.tile([P, RPD * KI * O], BF16, tag="eps")
                rings = [nc.sync, nc.scalar, nc.gpsimd][:N_RINGS]
                eng = rings[(b0 // RPD) % len(rings)]
                eng.dma_start(
                    out=eps_t[:].rearrange("p (r ki o) -> p r ki o",
                                           r=RPD, ki=KI),
                    in_=epsT_s.ap()[b0:b0 + RPD].rearrange(
                        "r (p ki) o -> p r ki o", p=P),
                )

                # t' = epsT * stdT, bf16 2x mode, SREP rows per tensor_tensor
                t_p = tp_pool.tile([P, RPD * KI * O], BF16, tag="tp")
                for s0 in range(0, RPD, SREP):
                    sl = slice(s0 * KI * O, (s0 + SREP) * KI * O)
                    nc.vector.tensor_tensor(out=t_p[:, sl], in0=eps_t[:, sl],
                                            in1=stdT_bf[:],
                                            op=mybir.AluOpType.mult)

                for r in range(RPD):
                    b = b0 + r
                    prow = prow_st[b % 4].ap()[:1, :]
                    for k in range(KI):
                        nc.tensor.matmul(
                            out=prow,
                            lhsT=xT_bf[:, k * B + b: k * B + b + 1],
                            rhs=t_p[:, (r * KI + k) * O:(r * KI + k + 1) * O],
                            start=(k == 0), stop=(k == KI - 1),
                        )

                    ci = b % STAGE_CHUNK
                    if ci == 0:
                        stage = stage_pool.tile([1, STAGE_CHUNK * O], F32,
                                                tag="stage")
                    nc.scalar.copy(
                        out=stage[0:1, ci * O:(ci + 1) * O], in_=prow)
                    if ci == STAGE_CHUNK - 1:
                        g0 = b - (STAGE_CHUNK - 1)
                        nc.gpsimd.dma_start(
                            out=G[g0:g0 + STAGE_CHUNK, :],
                            in_=stage[0:1, :],
                        )
                    if b % 32 == 31:
                        f0 = b - 31
                        nc.vector.tensor_tensor(
                            out=final_sb[f0:f0 + 32, :],
                            in0=G[f0:f0 + 32, :],
                            in1=U[f0:f0 + 32, :],
                            op=mybir.AluOpType.add,
                        )
                        nc.gpsimd.dma_start(
                            out=out_s.ap()[f0:f0 + 32, :],
                            in_=final_sb[f0:f0 + 32, :],
                        )

            repeat = int(os.environ.get("KERNEL_REPEAT", "0"))
            if repeat > 1:
                with tc.For_i(0, repeat, 1):
                    emit_main()
            else:
                emit_main()

    nc.compile()
    return nc


_NC = None


def _get_program():
    global _NC
    if _NC is None:
        _NC = _build_program()
    return _NC


def _prep_full(inputs):
    """Host-side layout/precision prep shared by kernel() and test harnesses."""
    x = np.asarray(inputs["x"], dtype=np.float32)
    eps_w = np.asarray(inputs["eps_w"], dtype=np.float32)
    eps_b = np.asarray(inputs["eps_b"], dtype=np.float32)
    w_mu = np.asarray(inputs["weight_mu"], dtype=np.float32)
    w_lv = np.asarray(inputs["weight_logvar"], dtype=np.float32)
    b_mu = np.asarray(inputs["bias_mu"], dtype=np.float32).reshape(1, O)
    b_lv = np.asarray(inputs["bias_logvar"], dtype=np.float32).reshape(1, O)

    eps_bf = eps_w.astype(ml_dtypes.bfloat16)          # (B_FULL, O, I)
    xT = np.ascontiguousarray(x.T)                     # (I, B_FULL)
    muT = np.ascontiguousarray(w_mu.T)                 # (I, O)
    lvT = np.ascontiguousarray(w_lv.T)                 # (I, O)
    return eps_bf, xT, muT, lvT, eps_b, b_mu, b_lv


def _core_maps(eps_bf, xT, muT, lvT, eps_b, b_mu, b_lv):
    in_maps = []
    for ci in range(N_CORES):
        sl = slice(ci * B, (ci + 1) * B)
        in_maps.append({
            "epsT_s": np.ascontiguousarray(
                eps_bf[sl].transpose(0, 2, 1)),       # (B, I, O) bf16
            "xT_s": np.ascontiguousarray(xT[:, sl]),  # (I, B)
            "muT_d": muT,
            "lvT_d": lvT,
            "eps_b_s": np.ascontiguousarray(eps_b[sl]),
            "b_mu": b_mu,
            "b_lv": b_lv,
        })
    return in_maps


def kernel(**inputs) -> np.ndarray:
    in_maps = _core_maps(*_prep_full(inputs))
    nc = _get_program()
    res = run_bass_kernel_spmd(nc, in_maps, core_ids=list(range(N_CORES)))
    out = np.concatenate([res.results[ci]["out_s"] for ci in range(N_CORES)],
                         axis=0)
    return out.astype(np.float32)
